# revision 4
# baseline (speedup 1.0000x reference)
"""Trainium2 Bass kernel: 2-layer GCN (PyG-style GCNConv x2) on 8 NeuronCores.

Strategy:
  - Nodes sharded contiguously across 8 cores (12500 rows each).
  - Per layer: dense h' = (x @ W) * dinv[row] computed on the owning core,
    AllGather h' to every core (51MB replica), then per-core sparse
    aggregation over its in-edges:
      gather h'[src] rows via dma_gather (int16 idx -> 4 src blocks of 25000),
      scatter-add via one-hot matmul into PSUM per 128-dst tile,
      bias added as rank-1 matmul outer(sqrt(deg), b),
      eviction scaled by dinv[dst] on the scalar engine.
  - The per-edge norm dinv[src]*dinv[dst] is folded into the two node-level
    scalings, so no per-edge multiply exists anywhere.
"""

import os
import sys

for _p in ("/opt/trn_rl_repo",):
    if _p not in sys.path:
        sys.path.append(_p)

import numpy as np

import concourse.bacc as bacc
import concourse.bass as bass
import concourse.mybir as mybir
import concourse.tile as tile
from concourse.bass_utils import run_bass_kernel_spmd

F32 = mybir.dt.float32
F16 = mybir.dt.float16
I16 = mybir.dt.int16
AF = mybir.ActivationFunctionType
ALU = mybir.AluOpType

N_NODES = 100000
D = 128
NCORES = 8
TILE = 128


def _ceil_div(a, b):
    return (a + b - 1) // b


class Plan:
    """Core-uniform structure tables derived from the edge index."""

    def __init__(self, n_nodes, edge_index, group_tiles=4):
        self.n = n_nodes
        self.ns = n_nodes // NCORES            # nodes per core
        self.nt = _ceil_div(self.ns, TILE)     # dst tiles per core
        self.last_w = self.ns - (self.nt - 1) * TILE
        self.nblk = _ceil_div(n_nodes, 32000)  # src blocks (int16 idx limit)
        self.blk = _ceil_div(n_nodes, self.nblk)
        self.G = group_tiles

        src = np.concatenate([edge_index[0], np.arange(n_nodes, dtype=np.int64)])
        dst = np.concatenate([edge_index[1], np.arange(n_nodes, dtype=np.int64)])
        deg = np.bincount(dst, minlength=n_nodes).astype(np.float32)
        self.dinv = deg ** -0.5
        self.sdeg = np.sqrt(deg)

        core = dst // self.ns
        tloc = (dst % self.ns) // TILE
        blk = src // self.blk
        key = (core * self.nt + tloc) * self.nblk + blk
        order = np.lexsort((src, key))
        self.src_s = src[order]
        self.doff_s = ((dst % self.ns) % TILE)[order]
        counts = np.bincount(key, minlength=NCORES * self.nt * self.nblk)
        self.counts = counts.reshape(NCORES, self.nt, self.nblk)
        # segment start offsets into src_s per (core, tile, blk)
        self.seg_off = np.zeros(NCORES * self.nt * self.nblk + 1, dtype=np.int64)
        np.cumsum(counts, out=self.seg_off[1:])

        # chunks per (tile, blk): shared across cores
        self.CT = _ceil_div(self.counts, TILE).max(axis=0)  # [nt, nblk]

        # tile groups
        self.groups = [list(range(g, min(g + self.G, self.nt)))
                       for g in range(0, self.nt, self.G)]

        # per (group, blk): chunk count and the (tile, n_chunks) layout
        self.gb_chunks = []   # [g][b] -> list of (tile, CT[t][b])
        self.gb_C = []        # [g][b] -> total chunks
        for tiles in self.groups:
            row_l, row_c = [], []
            for b in range(self.nblk):
                lay = [(t, int(self.CT[t, b])) for t in tiles if self.CT[t, b] > 0]
                row_l.append(lay)
                row_c.append(sum(c for _, c in lay))
            self.gb_chunks.append(row_l)
            self.gb_C.append(row_c)

        # column offsets in the concatenated idx / dstoff DRAM buffers
        self.idx_col = []     # [g][b] -> start col in idx buffer (int16, /16 wrap)
        self.ch_col = []      # [g] -> start chunk col in dstoff buffer
        ic = 0
        cc = 0
        for g in range(len(self.groups)):
            self.ch_col.append(cc)
            row = []
            for b in range(self.nblk):
                row.append(ic)
                ic += self.gb_C[g][b] * (TILE // 16)
                cc += self.gb_C[g][b]
            self.idx_col.append(row)
        self.idx_cols = ic
        self.ch_cols = cc

    def core_inputs(self, c):
        """Build idx (int16 [128, idx_cols]) and dstoff (f32 [128, ch_cols])."""
        idx = np.zeros((16, self.idx_cols), dtype=np.int16)
        doff = np.full((128, self.ch_cols), -1.0, dtype=np.float32)
        for g, tiles in enumerate(self.groups):
            ch = self.ch_col[g]
            for b in range(self.nblk):
                icol = self.idx_col[g][b]
                for (t, nch) in self.gb_chunks[g][b]:
                    cnt = int(self.counts[c, t, b])
                    o = self.seg_off[(c * self.nt + t) * self.nblk + b]
                    nslots = nch * TILE
                    a = np.zeros(nslots, dtype=np.int16)
                    s = self.src_s[o:o + cnt]
                    shard = s // self.ns
                    r = s % self.ns
                    ntl = _ceil_div(self.ns, 128)
                    vrow = (shard - 2 * b) * (ntl * 128) + (r % 128) * ntl + r // 128
                    a[:cnt] = vrow.astype(np.int16)
                    idx[:, icol:icol + nch * 8] = a.reshape(nch * 8, 16).T
                    dv = np.full(nslots, -1.0, dtype=np.float32)
                    dv[:cnt] = self.doff_s[o:o + cnt].astype(np.float32)
                    doff[:, ch:ch + nch] = dv.reshape(nch, 128).T
                    icol += nch * 8
                    ch += nch
        idx_full = np.tile(idx, (8, 1))
        return idx_full, doff


def _build(plan, stage="full"):
    """Build the SPMD bass program (shared by all 8 cores)."""
    n, ns, nt, nblk, blk = plan.n, plan.ns, plan.nt, plan.nblk, plan.blk
    nc = bacc.Bacc("TRN2", target_bir_lowering=False, debug=False,
                   num_devices=NCORES, num_swdge_queues=4)

    xT = nc.dram_tensor("xT", [D, ns], F16, kind="ExternalInput").ap()
    wts = nc.dram_tensor("wts", [D, 2 * D], F16, kind="ExternalInput").ap()
    consts = nc.dram_tensor("consts", [D, 2 * D], F32, kind="ExternalInput").ap()
    brow = nc.dram_tensor("brow", [1, 2 * D], F16, kind="ExternalInput").ap()
    dinv_c = nc.dram_tensor("dinv_c", [D, nt], F32, kind="ExternalInput").ap()
    sdeg_r = nc.dram_tensor("sdeg_r", [1, nt * TILE], F16, kind="ExternalInput").ap()
    idx_d = nc.dram_tensor("idx", [D, plan.idx_cols], I16, kind="ExternalInput").ap()
    doff_d = nc.dram_tensor("doff", [D, plan.ch_cols], F32, kind="ExternalInput").ap()
    out_d = nc.dram_tensor("out", [nt * D, D], F16, kind="ExternalOutput").ap()

    # h stored in "vrow" tile layout: vrow u = p * nt + t holds the fp16
    # row of node t*128+p (within its shard). One contiguous DMA per layer
    # writes it, and gathers index vrows directly.
    hb = [nc.dram_tensor(f"h{i}b", [nt * D, D], F16).ap() for i in range(2)]
    hf = [nc.dram_tensor(f"h{i}f", [NCORES * nt * D, D], F16,
                         addr_space="Shared").ap()
          for i in range(2)]

    max_C = max(sum(plan.gb_C[g]) for g in range(len(plan.groups)))
    max_icols = max(sum(plan.gb_C[g]) * 8 for g in range(len(plan.groups)))

    with tile.TileContext(nc) as tc:
        with (
            tc.tile_pool(name="const", bufs=1) as cpool,
            tc.tile_pool(name="stage", bufs=5) as spool,
            tc.tile_pool(name="oh", bufs=5) as ohpool,
            tc.tile_pool(name="hbig", bufs=2) as bpool,
            tc.tile_pool(name="ev", bufs=4) as evpool,
            tc.tile_pool(name="acc", bufs=4, space="PSUM") as accpool,
            tc.tile_pool(name="ptr", bufs=2, space="PSUM") as trpool,
            tc.tile_pool(name="pd", bufs=2, space="PSUM") as pdpool,
        ):
            w_sb = cpool.tile([D, 2 * D], F16, tag="w")
            nc.sync.dma_start(out=w_sb[:], in_=wts[:])
            co_sb = cpool.tile([D, 2 * D], F32, tag="co")
            nc.sync.dma_start(out=co_sb[:], in_=consts[:])
            br_sb = cpool.tile([1, 2 * D], F16, tag="br")
            nc.sync.dma_start(out=br_sb[:], in_=brow[:])
            dv_sb = cpool.tile([D, nt], F32, tag="dv")
            nc.sync.dma_start(out=dv_sb[:], in_=dinv_c[:])
            sd_sb = cpool.tile([1, nt * TILE], F16, tag="sd")
            nc.sync.dma_start(out=sd_sb[:], in_=sdeg_r[:])

            xall = cpool.tile([D, ns], F16, tag="xall")
            nc.sync.dma_start(out=xall[:], in_=xT[:])
            idx_all = cpool.tile([D, plan.idx_cols], I16, tag="idxall")
            nc.sync.dma_start(out=idx_all[:], in_=idx_d[:])
            do_all = cpool.tile([D, plan.ch_cols], F32, tag="doall")
            nc.sync.dma_start(out=do_all[:], in_=doff_d[:])

            W1 = w_sb[:, 0:D]
            W2 = w_sb[:, D:2 * D]
            iota = co_sb[:, 0:D]
            ident = co_sb[:, D:2 * D]

            def tw(t):
                return TILE if t < nt - 1 else plan.last_w

            # ---- layer-1 dense: h0' = (x @ W1) * dinv ----
            hbig0 = bpool.tile([D, nt, D], F16, tag="hbig", name="hbig0")
            for t in range(nt):
                w = tw(t)
                pd = pdpool.tile([TILE, D], F32, tag="pd")
                nc.tensor.matmul(pd[:w, :], lhsT=xall[:, t * TILE:t * TILE + w],
                                 rhs=W1, start=True, stop=True)
                nc.scalar.activation(hbig0[:w, t, :], pd[:w, :], AF.Copy,
                                     scale=dv_sb[:w, t:t + 1])
            nc.sync.dma_start(out=hb[0][:], in_=hbig0[:])

            # ---- sparse layer (templated over layer index) ----
            max_Cgb = max((plan.gb_C[g][b] for g in range(len(plan.groups))
                           for b in range(nblk)), default=1)
            GMAX = 8  # dma_gather caps at ~1024 idxs (16KB desc ring)

            qctr = [0]

            VR = 2 * nt * D  # vrows per 2-shard block

            def sparse_layer(li, dense_big=None, out_big=None):
                src_full = hf[li]
                for g, tiles in enumerate(plan.groups):
                    Ctot = sum(plan.gb_C[g])
                    if Ctot == 0:
                        continue
                    accs = {}
                    started = set()
                    for t in tiles:
                        accs[t] = accpool.tile([TILE, D], F32, tag="acc", name=f"acc_t{t}")

                    gco = 0  # chunk offset within the group's doff columns
                    for b in range(nblk):
                        Cgb = plan.gb_C[g][b]
                        if Cgb == 0:
                            continue
                        ic0 = plan.idx_col[g][b]
                        stg = spool.tile([D, max_Cgb, TILE], F16, tag="stage")
                        oh_sb = ohpool.tile([D, max_Cgb, TILE], F16, tag="oh")
                        nc.vector.scalar_tensor_tensor(
                            out=oh_sb[:, :Cgb, :],
                            in0=do_all[:, plan.ch_col[g] + gco:
                                       plan.ch_col[g] + gco + Cgb].unsqueeze(2)
                                .broadcast_to([D, Cgb, TILE]),
                            scalar=1.0,
                            in1=iota.unsqueeze(1).broadcast_to([D, Cgb, TILE]),
                            op0=ALU.mult,
                            op1=ALU.is_equal,
                        )
                        for c0 in range(0, Cgb, GMAX):
                            cn = min(GMAX, Cgb - c0)
                            nc.gpsimd.dma_gather(
                                stg[:, c0:c0 + cn, :],
                                src_full[b * VR:(b + 1) * VR, :],
                                idx_all[:, ic0 + c0 * 8:ic0 + (c0 + cn) * 8],
                                cn * TILE,
                                cn * TILE,
                                D,
                                queue_num=qctr[0] % 4,
                            )
                            qctr[0] += 1
                        k = 0
                        for (t, nch) in plan.gb_chunks[g][b]:
                            for _ in range(nch):
                                nc.tensor.matmul(
                                    accs[t][:], lhsT=oh_sb[:, k, :],
                                    rhs=stg[:, k, :],
                                    start=(t not in started), stop=False)
                                started.add(t)
                                k += 1
                        gco += Cgb

                    for t in tiles:
                        if t not in started:
                            continue
                        w = tw(t)
                        acc = accs[t]
                        # bias as rank-1: outer(sqrt(deg), b); sdeg rows
                        # beyond the tile width are zero-padded on the host
                        nc.tensor.matmul(
                            acc[:],
                            lhsT=sd_sb[:, t * TILE:(t + 1) * TILE],
                            rhs=br_sb[:, li * D:(li + 1) * D],
                            start=False, stop=True)
                        if li == 1:
                            nc.scalar.activation(out_big[:w, t, :], acc[:w, :],
                                                 AF.Copy,
                                                 scale=dv_sb[:w, t:t + 1])
                            continue
                        ev = evpool.tile([TILE, D], F32, tag="ev")
                        nc.scalar.activation(ev[:w, :], acc[:w, :], AF.Copy,
                                             scale=dv_sb[:w, t:t + 1])
                        if li == 0:
                            # fused layer-2 dense: h1' = (out1 @ W2) * dinv
                            ptr = trpool.tile([D, TILE], F32, tag="ptr")
                            nc.tensor.transpose(ptr[:, :w], ev[:w, :],
                                                ident[:w, :w])
                            trs = evpool.tile([D, TILE], F16, tag="trs")
                            nc.vector.tensor_copy(trs[:, :w], ptr[:, :w])
                            pd = pdpool.tile([TILE, D], F32, tag="pd")
                            nc.tensor.matmul(pd[:w, :], lhsT=trs[:, :w], rhs=W2,
                                             start=True, stop=True)
                            nc.scalar.activation(dense_big[:w, t, :], pd[:w, :],
                                                 AF.Copy,
                                                 scale=dv_sb[:w, t:t + 1])

            nc.gpsimd.collective_compute(
                "AllGather", ALU.bypass,
                replica_groups=[list(range(NCORES))],
                ins=[hb[0].opt()], outs=[hf[0].opt()])
            hbig1 = bpool.tile([D, nt, D], F16, tag="hbig", name="hbig1")
            sparse_layer(0, dense_big=hbig1)
            nc.sync.dma_start(out=hb[1][:], in_=hbig1[:])
            nc.gpsimd.collective_compute(
                "AllGather", ALU.bypass,
                replica_groups=[list(range(NCORES))],
                ins=[hb[1].opt()], outs=[hf[1].opt()])
            obig = bpool.tile([D, nt, D], F16, tag="hbig", name="obig")
            sparse_layer(1, out_big=obig)
            nc.sync.dma_start(out=out_d[:], in_=obig[:])

    nc.compile()
    return nc


def _install_ntff_hook():
    """antenv.axon_hooks is absent in this image; synthesize it and register
    the ctypes NTFF profile hook from the boot module."""
    import types
    if "antenv.axon_hooks" in sys.modules:
        return
    try:
        from trn_agent_boot.trn_boot import _ntff_profile_via_ctypes
        hook = _ntff_profile_via_ctypes("/opt/axon/libaxon_pjrt.so")
    except Exception as e:
        print(f"[kernel] ntff hook unavailable: {e}", flush=True)
        hook = None
    mod = types.ModuleType("antenv.axon_hooks")
    mod._hook = hook
    mod.set_axon_ntff_profile_hook = lambda h: setattr(mod, "_hook", h)
    mod.get_axon_ntff_profile_hook = lambda: mod._hook
    sys.modules["antenv.axon_hooks"] = mod
    import antenv
    antenv.axon_hooks = mod


def _run(plan, x, W1, b1, W2, b2, trace=False, stage="full"):
    import time
    if trace:
        _install_ntff_hook()
    t0 = time.time()
    nc = _build(plan, stage=stage)
    t1 = time.time()
    if os.environ.get("GCN_VERBOSE"):
        print(f"[kernel] build+compile: {t1 - t0:.1f}s", flush=True)
    ns, nt = plan.ns, plan.nt
    iota_t = np.tile(np.arange(TILE, dtype=np.float32), (TILE, 1))
    ident_t = np.eye(TILE, dtype=np.float32)
    consts = np.concatenate([iota_t, ident_t], axis=1)
    wts = np.concatenate([W1, W2], axis=1).astype(np.float16)
    brow = np.concatenate([b1, b2]).astype(np.float16).reshape(1, 2 * D)

    in_maps = []
    for c in range(NCORES):
        lo, hi = c * ns, (c + 1) * ns
        dv = plan.dinv[lo:hi]
        # column t of dcol holds dinv[lo + t*128 : lo + (t+1)*128] (pad 1.0)
        dcol = np.ones((nt, TILE), dtype=np.float32)
        dcol.reshape(-1)[:ns] = dv
        dcol = np.ascontiguousarray(dcol.T)
        sdr = np.zeros((1, nt * TILE), dtype=np.float16)
        sdr[0, :ns] = plan.sdeg[lo:hi].astype(np.float16)
        idx, doff = plan.core_inputs(c)
        in_maps.append({
            "xT": np.ascontiguousarray(x[lo:hi].T.astype(np.float16)),
            "wts": wts, "consts": consts, "brow": brow,
            "dinv_c": dcol, "sdeg_r": sdr,
            "idx": idx, "doff": doff,
        })
    t2 = time.time()
    res = run_bass_kernel_spmd(nc, in_maps, core_ids=list(range(NCORES)),
                               trace=trace)
    if os.environ.get("GCN_VERBOSE"):
        print(f"[kernel] prep inputs: {t2 - t1:.1f}s, run: {time.time() - t2:.1f}s",
              flush=True)
    nt = plan.nt
    parts = []
    for c in range(NCORES):
        buf = res.results[c]["out"]  # [nt*128, 128] f16, vrow u = p*nt + t
        o = buf.reshape(128, nt, 128).transpose(1, 0, 2).reshape(nt * 128, 128)
        parts.append(o[:plan.ns].astype(np.float32))
    out = np.concatenate(parts, axis=0)
    return out, res


def kernel(x, edge_index, W1, b1, W2, b2):
    plan = Plan(x.shape[0], np.asarray(edge_index))
    out, _ = _run(plan, np.asarray(x), np.asarray(W1), np.asarray(b1),
                  np.asarray(W2), np.asarray(b2))
    return out



# revision 5
# speedup vs baseline: 1.1290x; 1.1290x over previous
"""Trainium2 Bass kernel: 2-layer GCN (PyG-style GCNConv x2) on 8 NeuronCores.

Strategy (tuned for the dma_gather roofline):
  - Nodes sharded contiguously across 8 cores (12500 rows each).
  - fp16 data path: dense h' = (x @ W) * dinv[row] computed in fp16 on the
    owning core into a "vrow"-tiled layout (vrow u = p*nt + t holds node
    t*128+p), written with one contiguous DMA, then AllGather'd (3.2MB/core).
  - Per-core sparse aggregation over its in-edges:
      gather h'[src] rows via dma_gather with int16 vrow indices into
      2-shard blocks, cycling 4 SWDGE queues (each queue has its own
      descriptor ring, so descriptor generation overlaps SDMA drain and
      each SDMA engine interleaves 4 packet streams against HBM latency);
      scatter-add via fp16 one-hot matmul into fp32 PSUM per 128-dst tile;
      bias as rank-1 matmul outer(sqrt(deg), b);
      eviction scaled by dinv[dst] on the scalar engine.
  - Layer-2 dense is fused into the layer-1 eviction (PE transpose + W2).
  - idx/doff tables load once; deep stage/oh pools keep ~14 SDMA engines
    busy.  Output leaves in vrow-tiled fp16 and is unpacked on the host.
  - The per-edge norm dinv[src]*dinv[dst] is folded into the two node-level
    scalings, so no per-edge multiply exists anywhere.

Empirical notes (HW-measured):
  - 256B gather descriptors: ~36ns/desc isolated, ~52ns/desc in-kernel.
  - Padding rows to 512B descriptors doubles HBM bytes and loses.
  - single_packet=False and >1024-idx calls (ring overflow) both lose.
"""

import os
import sys

for _p in ("/opt/trn_rl_repo",):
    if _p not in sys.path:
        sys.path.append(_p)

import numpy as np

import concourse.bacc as bacc
import concourse.bass as bass
import concourse.mybir as mybir
import concourse.tile as tile
from concourse.bass_utils import run_bass_kernel_spmd

F32 = mybir.dt.float32
F16 = mybir.dt.float16
I16 = mybir.dt.int16
AF = mybir.ActivationFunctionType
ALU = mybir.AluOpType

N_NODES = 100000
D = 128
NCORES = 8
TILE = 128


def _ceil_div(a, b):
    return (a + b - 1) // b


class Plan:
    """Core-uniform structure tables derived from the edge index."""

    def __init__(self, n_nodes, edge_index, group_tiles=4):
        self.n = n_nodes
        self.ns = n_nodes // NCORES            # nodes per core
        self.nt = _ceil_div(self.ns, TILE)     # dst tiles per core
        self.last_w = self.ns - (self.nt - 1) * TILE
        self.nblk = _ceil_div(n_nodes, 32000)  # src blocks (int16 idx limit)
        self.blk = _ceil_div(n_nodes, self.nblk)
        self.G = group_tiles

        src = np.concatenate([edge_index[0], np.arange(n_nodes, dtype=np.int64)])
        dst = np.concatenate([edge_index[1], np.arange(n_nodes, dtype=np.int64)])
        deg = np.bincount(dst, minlength=n_nodes).astype(np.float32)
        self.dinv = deg ** -0.5
        self.sdeg = np.sqrt(deg)

        core = dst // self.ns
        tloc = (dst % self.ns) // TILE
        blk = src // self.blk
        key = (core * self.nt + tloc) * self.nblk + blk
        order = np.lexsort((src, key))
        self.src_s = src[order]
        self.doff_s = ((dst % self.ns) % TILE)[order]
        counts = np.bincount(key, minlength=NCORES * self.nt * self.nblk)
        self.counts = counts.reshape(NCORES, self.nt, self.nblk)
        # segment start offsets into src_s per (core, tile, blk)
        self.seg_off = np.zeros(NCORES * self.nt * self.nblk + 1, dtype=np.int64)
        np.cumsum(counts, out=self.seg_off[1:])

        # chunks per (tile, blk): shared across cores
        self.CT = _ceil_div(self.counts, TILE).max(axis=0)  # [nt, nblk]

        # tile groups
        self.groups = [list(range(g, min(g + self.G, self.nt)))
                       for g in range(0, self.nt, self.G)]

        # per (group, blk): chunk count and the (tile, n_chunks) layout
        self.gb_chunks = []   # [g][b] -> list of (tile, CT[t][b])
        self.gb_C = []        # [g][b] -> total chunks
        for tiles in self.groups:
            row_l, row_c = [], []
            for b in range(self.nblk):
                lay = [(t, int(self.CT[t, b])) for t in tiles if self.CT[t, b] > 0]
                row_l.append(lay)
                row_c.append(sum(c for _, c in lay))
            self.gb_chunks.append(row_l)
            self.gb_C.append(row_c)

        # column offsets in the concatenated idx / dstoff DRAM buffers
        self.idx_col = []     # [g][b] -> start col in idx buffer (int16, /16 wrap)
        self.ch_col = []      # [g] -> start chunk col in dstoff buffer
        ic = 0
        cc = 0
        for g in range(len(self.groups)):
            self.ch_col.append(cc)
            row = []
            for b in range(self.nblk):
                row.append(ic)
                ic += self.gb_C[g][b] * (TILE // 16)
                cc += self.gb_C[g][b]
            self.idx_col.append(row)
        self.idx_cols = ic
        self.ch_cols = cc

    def core_inputs(self, c):
        """Build idx (int16 [128, idx_cols]) and dstoff (f32 [128, ch_cols])."""
        idx = np.zeros((16, self.idx_cols), dtype=np.int16)
        doff = np.full((128, self.ch_cols), -1.0, dtype=np.float32)
        for g, tiles in enumerate(self.groups):
            ch = self.ch_col[g]
            for b in range(self.nblk):
                icol = self.idx_col[g][b]
                for (t, nch) in self.gb_chunks[g][b]:
                    cnt = int(self.counts[c, t, b])
                    o = self.seg_off[(c * self.nt + t) * self.nblk + b]
                    nslots = nch * TILE
                    a = np.zeros(nslots, dtype=np.int16)
                    s = self.src_s[o:o + cnt]
                    shard = s // self.ns
                    r = s % self.ns
                    ntl = _ceil_div(self.ns, 128)
                    vrow = (shard - 2 * b) * (ntl * 128) + (r % 128) * ntl + r // 128
                    a[:cnt] = vrow.astype(np.int16)
                    idx[:, icol:icol + nch * 8] = a.reshape(nch * 8, 16).T
                    dv = np.full(nslots, -1.0, dtype=np.float32)
                    dv[:cnt] = self.doff_s[o:o + cnt].astype(np.float32)
                    doff[:, ch:ch + nch] = dv.reshape(nch, 128).T
                    icol += nch * 8
                    ch += nch
        idx_full = np.tile(idx, (8, 1))
        return idx_full, doff


def _build(plan, stage="full"):
    """Build the SPMD bass program (shared by all 8 cores)."""
    n, ns, nt, nblk, blk = plan.n, plan.ns, plan.nt, plan.nblk, plan.blk
    nc = bacc.Bacc("TRN2", target_bir_lowering=False, debug=False,
                   num_devices=NCORES, num_swdge_queues=4)

    xT = nc.dram_tensor("xT", [D, ns], F16, kind="ExternalInput").ap()
    wts = nc.dram_tensor("wts", [D, 2 * D], F16, kind="ExternalInput").ap()
    consts = nc.dram_tensor("consts", [D, 2 * D], F32, kind="ExternalInput").ap()
    brow = nc.dram_tensor("brow", [1, 2 * D], F16, kind="ExternalInput").ap()
    dinv_c = nc.dram_tensor("dinv_c", [D, nt], F32, kind="ExternalInput").ap()
    sdeg_r = nc.dram_tensor("sdeg_r", [1, nt * TILE], F16, kind="ExternalInput").ap()
    idx_d = nc.dram_tensor("idx", [D, plan.idx_cols], I16, kind="ExternalInput").ap()
    doff_d = nc.dram_tensor("doff", [D, plan.ch_cols], F32, kind="ExternalInput").ap()
    out_d = nc.dram_tensor("out", [nt * D, D], F16, kind="ExternalOutput").ap()

    # h stored in "vrow" tile layout: vrow u = p * nt + t holds the fp16
    # row of node t*128+p (within its shard). One contiguous DMA per layer
    # writes it, and gathers index vrows directly.
    hb = [nc.dram_tensor(f"h{i}b", [nt * D, D], F16).ap() for i in range(2)]
    hf = [nc.dram_tensor(f"h{i}f", [NCORES * nt * D, D], F16,
                         addr_space="Shared").ap()
          for i in range(2)]

    max_C = max(sum(plan.gb_C[g]) for g in range(len(plan.groups)))
    max_icols = max(sum(plan.gb_C[g]) * 8 for g in range(len(plan.groups)))

    with tile.TileContext(nc) as tc:
        with (
            tc.tile_pool(name="const", bufs=1) as cpool,
            tc.tile_pool(name="stage", bufs=5) as spool,
            tc.tile_pool(name="oh", bufs=5) as ohpool,
            tc.tile_pool(name="hbig", bufs=2) as bpool,
            tc.tile_pool(name="ev", bufs=4) as evpool,
            tc.tile_pool(name="acc", bufs=4, space="PSUM") as accpool,
            tc.tile_pool(name="ptr", bufs=2, space="PSUM") as trpool,
            tc.tile_pool(name="pd", bufs=2, space="PSUM") as pdpool,
        ):
            w_sb = cpool.tile([D, 2 * D], F16, tag="w")
            nc.sync.dma_start(out=w_sb[:], in_=wts[:])
            co_sb = cpool.tile([D, 2 * D], F32, tag="co")
            nc.sync.dma_start(out=co_sb[:], in_=consts[:])
            br_sb = cpool.tile([1, 2 * D], F16, tag="br")
            nc.sync.dma_start(out=br_sb[:], in_=brow[:])
            dv_sb = cpool.tile([D, nt], F32, tag="dv")
            nc.sync.dma_start(out=dv_sb[:], in_=dinv_c[:])
            sd_sb = cpool.tile([1, nt * TILE], F16, tag="sd")
            nc.sync.dma_start(out=sd_sb[:], in_=sdeg_r[:])

            xall = cpool.tile([D, ns], F16, tag="xall")
            nc.sync.dma_start(out=xall[:], in_=xT[:])
            idx_all = cpool.tile([D, plan.idx_cols], I16, tag="idxall")
            nc.sync.dma_start(out=idx_all[:], in_=idx_d[:])
            do_all = cpool.tile([D, plan.ch_cols], F32, tag="doall")
            nc.sync.dma_start(out=do_all[:], in_=doff_d[:])

            W1 = w_sb[:, 0:D]
            W2 = w_sb[:, D:2 * D]
            iota = co_sb[:, 0:D]
            ident = co_sb[:, D:2 * D]

            def tw(t):
                return TILE if t < nt - 1 else plan.last_w

            # ---- layer-1 dense: h0' = (x @ W1) * dinv ----
            hbig0 = bpool.tile([D, nt, D], F16, tag="hbig", name="hbig0")
            for t in range(nt):
                w = tw(t)
                pd = pdpool.tile([TILE, D], F32, tag="pd")
                nc.tensor.matmul(pd[:w, :], lhsT=xall[:, t * TILE:t * TILE + w],
                                 rhs=W1, start=True, stop=True)
                nc.scalar.activation(hbig0[:w, t, :], pd[:w, :], AF.Copy,
                                     scale=dv_sb[:w, t:t + 1])
            nc.sync.dma_start(out=hb[0][:], in_=hbig0[:])

            # ---- sparse layer (templated over layer index) ----
            max_Cgb = max((plan.gb_C[g][b] for g in range(len(plan.groups))
                           for b in range(nblk)), default=1)
            GMAX = 8  # dma_gather caps at ~1024 idxs (16KB desc ring)

            qctr = [0]

            VR = 2 * nt * D  # vrows per 2-shard block

            def sparse_layer(li, dense_big=None, out_big=None):
                src_full = hf[li]
                for g, tiles in enumerate(plan.groups):
                    Ctot = sum(plan.gb_C[g])
                    if Ctot == 0:
                        continue
                    accs = {}
                    started = set()
                    for t in tiles:
                        accs[t] = accpool.tile([TILE, D], F32, tag="acc", name=f"acc_t{t}")

                    gco = 0  # chunk offset within the group's doff columns
                    for b in range(nblk):
                        Cgb = plan.gb_C[g][b]
                        if Cgb == 0:
                            continue
                        ic0 = plan.idx_col[g][b]
                        stg = spool.tile([D, max_Cgb, TILE], F16, tag="stage")
                        oh_sb = ohpool.tile([D, max_Cgb, TILE], F16, tag="oh")
                        nc.vector.scalar_tensor_tensor(
                            out=oh_sb[:, :Cgb, :],
                            in0=do_all[:, plan.ch_col[g] + gco:
                                       plan.ch_col[g] + gco + Cgb].unsqueeze(2)
                                .broadcast_to([D, Cgb, TILE]),
                            scalar=1.0,
                            in1=iota.unsqueeze(1).broadcast_to([D, Cgb, TILE]),
                            op0=ALU.mult,
                            op1=ALU.is_equal,
                        )
                        for c0 in range(0, Cgb, GMAX):
                            cn = min(GMAX, Cgb - c0)
                            nc.gpsimd.dma_gather(
                                stg[:, c0:c0 + cn, :],
                                src_full[b * VR:(b + 1) * VR, :],
                                idx_all[:, ic0 + c0 * 8:ic0 + (c0 + cn) * 8],
                                cn * TILE,
                                cn * TILE,
                                D,
                                queue_num=qctr[0] % 4,
                            )
                            qctr[0] += 1
                        k = 0
                        for (t, nch) in plan.gb_chunks[g][b]:
                            for _ in range(nch):
                                nc.tensor.matmul(
                                    accs[t][:], lhsT=oh_sb[:, k, :],
                                    rhs=stg[:, k, :],
                                    start=(t not in started), stop=False)
                                started.add(t)
                                k += 1
                        gco += Cgb

                    for t in tiles:
                        if t not in started:
                            continue
                        w = tw(t)
                        acc = accs[t]
                        # bias as rank-1: outer(sqrt(deg), b); sdeg rows
                        # beyond the tile width are zero-padded on the host
                        nc.tensor.matmul(
                            acc[:],
                            lhsT=sd_sb[:, t * TILE:(t + 1) * TILE],
                            rhs=br_sb[:, li * D:(li + 1) * D],
                            start=False, stop=True)
                        if li == 1:
                            nc.scalar.activation(out_big[:w, t, :], acc[:w, :],
                                                 AF.Copy,
                                                 scale=dv_sb[:w, t:t + 1])
                            continue
                        ev = evpool.tile([TILE, D], F32, tag="ev")
                        nc.scalar.activation(ev[:w, :], acc[:w, :], AF.Copy,
                                             scale=dv_sb[:w, t:t + 1])
                        if li == 0:
                            # fused layer-2 dense: h1' = (out1 @ W2) * dinv
                            ptr = trpool.tile([D, TILE], F32, tag="ptr")
                            nc.tensor.transpose(ptr[:, :w], ev[:w, :],
                                                ident[:w, :w])
                            trs = evpool.tile([D, TILE], F16, tag="trs")
                            nc.vector.tensor_copy(trs[:, :w], ptr[:, :w])
                            pd = pdpool.tile([TILE, D], F32, tag="pd")
                            nc.tensor.matmul(pd[:w, :], lhsT=trs[:, :w], rhs=W2,
                                             start=True, stop=True)
                            nc.scalar.activation(dense_big[:w, t, :], pd[:w, :],
                                                 AF.Copy,
                                                 scale=dv_sb[:w, t:t + 1])

            nc.gpsimd.collective_compute(
                "AllGather", ALU.bypass,
                replica_groups=[list(range(NCORES))],
                ins=[hb[0].opt()], outs=[hf[0].opt()])
            hbig1 = bpool.tile([D, nt, D], F16, tag="hbig", name="hbig1")
            sparse_layer(0, dense_big=hbig1)
            nc.sync.dma_start(out=hb[1][:], in_=hbig1[:])
            nc.gpsimd.collective_compute(
                "AllGather", ALU.bypass,
                replica_groups=[list(range(NCORES))],
                ins=[hb[1].opt()], outs=[hf[1].opt()])
            obig = bpool.tile([D, nt, D], F16, tag="hbig", name="obig")
            sparse_layer(1, out_big=obig)
            nc.sync.dma_start(out=out_d[:], in_=obig[:])

    nc.compile()
    return nc


def _install_ntff_hook():
    """antenv.axon_hooks is absent in this image; synthesize it and register
    the ctypes NTFF profile hook from the boot module."""
    import types
    if "antenv.axon_hooks" in sys.modules:
        return
    try:
        from trn_agent_boot.trn_boot import _ntff_profile_via_ctypes
        hook = _ntff_profile_via_ctypes("/opt/axon/libaxon_pjrt.so")
    except Exception as e:
        print(f"[kernel] ntff hook unavailable: {e}", flush=True)
        hook = None
    mod = types.ModuleType("antenv.axon_hooks")
    mod._hook = hook
    mod.set_axon_ntff_profile_hook = lambda h: setattr(mod, "_hook", h)
    mod.get_axon_ntff_profile_hook = lambda: mod._hook
    sys.modules["antenv.axon_hooks"] = mod
    import antenv
    antenv.axon_hooks = mod


def _run(plan, x, W1, b1, W2, b2, trace=False, stage="full"):
    import time
    if trace:
        _install_ntff_hook()
    t0 = time.time()
    nc = _build(plan, stage=stage)
    t1 = time.time()
    if os.environ.get("GCN_VERBOSE"):
        print(f"[kernel] build+compile: {t1 - t0:.1f}s", flush=True)
    ns, nt = plan.ns, plan.nt
    iota_t = np.tile(np.arange(TILE, dtype=np.float32), (TILE, 1))
    ident_t = np.eye(TILE, dtype=np.float32)
    consts = np.concatenate([iota_t, ident_t], axis=1)
    wts = np.concatenate([W1, W2], axis=1).astype(np.float16)
    brow = np.concatenate([b1, b2]).astype(np.float16).reshape(1, 2 * D)

    in_maps = []
    for c in range(NCORES):
        lo, hi = c * ns, (c + 1) * ns
        dv = plan.dinv[lo:hi]
        # column t of dcol holds dinv[lo + t*128 : lo + (t+1)*128] (pad 1.0)
        dcol = np.ones((nt, TILE), dtype=np.float32)
        dcol.reshape(-1)[:ns] = dv
        dcol = np.ascontiguousarray(dcol.T)
        sdr = np.zeros((1, nt * TILE), dtype=np.float16)
        sdr[0, :ns] = plan.sdeg[lo:hi].astype(np.float16)
        idx, doff = plan.core_inputs(c)
        in_maps.append({
            "xT": np.ascontiguousarray(x[lo:hi].T.astype(np.float16)),
            "wts": wts, "consts": consts, "brow": brow,
            "dinv_c": dcol, "sdeg_r": sdr,
            "idx": idx, "doff": doff,
        })
    t2 = time.time()
    res = run_bass_kernel_spmd(nc, in_maps, core_ids=list(range(NCORES)),
                               trace=trace)
    if os.environ.get("GCN_VERBOSE"):
        print(f"[kernel] prep inputs: {t2 - t1:.1f}s, run: {time.time() - t2:.1f}s",
              flush=True)
    nt = plan.nt
    parts = []
    for c in range(NCORES):
        buf = res.results[c]["out"]  # [nt*128, 128] f16, vrow u = p*nt + t
        o = buf.reshape(128, nt, 128).transpose(1, 0, 2).reshape(nt * 128, 128)
        parts.append(o[:plan.ns].astype(np.float32))
    out = np.concatenate(parts, axis=0)
    return out, res


def kernel(x, edge_index, W1, b1, W2, b2):
    plan = Plan(x.shape[0], np.asarray(edge_index))
    out, _ = _run(plan, np.asarray(x), np.asarray(W1), np.asarray(b1),
                  np.asarray(W2), np.asarray(b2))
    return out



# revision 6
# speedup vs baseline: 1.2071x; 1.0692x over previous
"""Trainium2 Bass kernel: 2-layer GCN (PyG-style GCNConv x2) on 8 NeuronCores.

Strategy (tuned for the dma_gather roofline):
  - Nodes sharded contiguously across 8 cores (12500 rows each).
  - fp16 data path: dense h' = (x @ W) * dinv[row] computed in fp16 on the
    owning core into a "vrow"-tiled layout (vrow u = p*nt + t holds node
    t*128+p), written with one contiguous DMA, then AllGather'd (3.2MB/core).
  - Per-core sparse aggregation over its in-edges:
      gather h'[src] rows via dma_gather with int16 vrow indices into
      2-shard blocks, cycling 4 SWDGE queues (each queue has its own
      descriptor ring, so descriptor generation overlaps SDMA drain and
      each SDMA engine interleaves 4 packet streams against HBM latency);
      scatter-add via fp16 one-hot matmul into fp32 PSUM per 128-dst tile;
      bias as rank-1 matmul outer(sqrt(deg), b);
      eviction scaled by dinv[dst] on the scalar engine.
  - Layer-2 dense is fused into the layer-1 eviction (PE transpose + W2).
  - idx/doff tables load once; deep stage/oh pools keep ~14 SDMA engines
    busy.  Output leaves in vrow-tiled fp16 and is unpacked on the host.
  - Gather calls are one-per-(tile,block) with trailing -1 index padding and
    a per-core runtime count register: the Q7 ucode trims trailing negatives,
    so the ~26% core-uniform slot padding costs no descriptors or gen time.
    (num_idxs_reg must equal the valid count - ring accounting reserves from
    the register, and a mismatch executes stale ring slots.)
  - The per-edge norm dinv[src]*dinv[dst] is folded into the two node-level
    scalings, so no per-edge multiply exists anywhere.

Empirical notes (HW-measured):
  - 256B gather descriptors: ~36ns/desc isolated, ~52ns/desc in-kernel.
  - Padding rows to 512B descriptors doubles HBM bytes and loses.
  - single_packet=False and >1024-idx calls (ring overflow) both lose.
"""

import os
import sys

for _p in ("/opt/trn_rl_repo",):
    if _p not in sys.path:
        sys.path.append(_p)

import numpy as np

import concourse.bacc as bacc
import concourse.bass as bass
import concourse.mybir as mybir
import concourse.tile as tile
from concourse.bass_utils import run_bass_kernel_spmd

F32 = mybir.dt.float32
F16 = mybir.dt.float16
I16 = mybir.dt.int16
AF = mybir.ActivationFunctionType
ALU = mybir.AluOpType

N_NODES = 100000
D = 128
NCORES = 8
TILE = 128


def _ceil_div(a, b):
    return (a + b - 1) // b


class Plan:
    """Core-uniform structure tables derived from the edge index."""

    def __init__(self, n_nodes, edge_index, group_tiles=4):
        self.n = n_nodes
        self.ns = n_nodes // NCORES            # nodes per core
        self.nt = _ceil_div(self.ns, TILE)     # dst tiles per core
        self.last_w = self.ns - (self.nt - 1) * TILE
        self.nblk = _ceil_div(n_nodes, 32000)  # src blocks (int16 idx limit)
        self.blk = _ceil_div(n_nodes, self.nblk)
        self.G = group_tiles

        src = np.concatenate([edge_index[0], np.arange(n_nodes, dtype=np.int64)])
        dst = np.concatenate([edge_index[1], np.arange(n_nodes, dtype=np.int64)])
        deg = np.bincount(dst, minlength=n_nodes).astype(np.float32)
        self.dinv = deg ** -0.5
        self.sdeg = np.sqrt(deg)

        core = dst // self.ns
        tloc = (dst % self.ns) // TILE
        blk = src // self.blk
        key = (core * self.nt + tloc) * self.nblk + blk
        order = np.lexsort((src, key))
        self.src_s = src[order]
        self.doff_s = ((dst % self.ns) % TILE)[order]
        counts = np.bincount(key, minlength=NCORES * self.nt * self.nblk)
        self.counts = counts.reshape(NCORES, self.nt, self.nblk)
        # segment start offsets into src_s per (core, tile, blk)
        self.seg_off = np.zeros(NCORES * self.nt * self.nblk + 1, dtype=np.int64)
        np.cumsum(counts, out=self.seg_off[1:])

        # chunks per (tile, blk): shared across cores
        self.CT = _ceil_div(self.counts, TILE).max(axis=0)  # [nt, nblk]

        # tile groups
        self.groups = [list(range(g, min(g + self.G, self.nt)))
                       for g in range(0, self.nt, self.G)]

        # per (group, blk): chunk count and the (tile, n_chunks) layout
        self.gb_chunks = []   # [g][b] -> list of (tile, CT[t][b])
        self.gb_C = []        # [g][b] -> total chunks
        for tiles in self.groups:
            row_l, row_c = [], []
            for b in range(self.nblk):
                lay = [(t, int(self.CT[t, b])) for t in tiles if self.CT[t, b] > 0]
                row_l.append(lay)
                row_c.append(sum(c for _, c in lay))
            self.gb_chunks.append(row_l)
            self.gb_C.append(row_c)

        # column offsets in the concatenated idx / dstoff DRAM buffers
        self.idx_col = []     # [g][b] -> start col in idx buffer (int16, /16 wrap)
        self.ch_col = []      # [g] -> start chunk col in dstoff buffer
        ic = 0
        cc = 0
        for g in range(len(self.groups)):
            self.ch_col.append(cc)
            row = []
            for b in range(self.nblk):
                row.append(ic)
                ic += self.gb_C[g][b] * (TILE // 16)
                cc += self.gb_C[g][b]
            self.idx_col.append(row)
        self.idx_cols = ic
        self.ch_cols = cc
        ncalls = sum((1 if ct <= 8 else (ct + 7) // 8)
                     for g in range(len(self.groups))
                     for b in range(self.nblk)
                     for _, ct in self.gb_chunks[g][b])
        self.n_calls_padded = ncalls + ((-ncalls) % 8)

    def call_counts(self, c):
        """Valid-idx count per gather call, in emission order (padded to 8)."""
        out = []
        for g in range(len(self.groups)):
            for b in range(self.nblk):
                if self.gb_C[g][b] == 0:
                    continue
                for (t, nch) in self.gb_chunks[g][b]:
                    cnt = int(self.counts[c, t, b])
                    for c0 in range(0, nch, 8):
                        cn = min(8, nch - c0)
                        out.append(min(max(cnt - c0 * TILE, 0), cn * TILE))
        pad = (-len(out)) % 8
        return np.array(out + [0] * pad, dtype=np.int32).reshape(1, -1)

    def core_inputs(self, c):
        """Build idx (int16 [128, idx_cols]) and dstoff (f32 [128, ch_cols])."""
        idx = np.zeros((16, self.idx_cols), dtype=np.int16)
        doff = np.full((128, self.ch_cols), -1.0, dtype=np.float16)
        for g, tiles in enumerate(self.groups):
            ch = self.ch_col[g]
            for b in range(self.nblk):
                icol = self.idx_col[g][b]
                for (t, nch) in self.gb_chunks[g][b]:
                    cnt = int(self.counts[c, t, b])
                    o = self.seg_off[(c * self.nt + t) * self.nblk + b]
                    nslots = nch * TILE
                    a = np.full(nslots, -1, dtype=np.int16)
                    s = self.src_s[o:o + cnt]
                    shard = s // self.ns
                    r = s % self.ns
                    ntl = _ceil_div(self.ns, 128)
                    vrow = (shard - 2 * b) * (ntl * 128) + (r % 128) * ntl + r // 128
                    a[:cnt] = vrow.astype(np.int16)
                    idx[:, icol:icol + nch * 8] = a.reshape(nch * 8, 16).T
                    dv = np.full(nslots, -1.0, dtype=np.float16)
                    dv[:cnt] = self.doff_s[o:o + cnt].astype(np.float16)
                    doff[:, ch:ch + nch] = dv.reshape(nch, 128).T
                    icol += nch * 8
                    ch += nch
        idx_full = np.tile(idx, (8, 1))
        return idx_full, doff


def _build(plan, stage="full"):
    """Build the SPMD bass program (shared by all 8 cores)."""
    n, ns, nt, nblk, blk = plan.n, plan.ns, plan.nt, plan.nblk, plan.blk
    nc = bacc.Bacc("TRN2", target_bir_lowering=False, debug=False,
                   num_devices=NCORES, num_swdge_queues=4)

    xT = nc.dram_tensor("xT", [D, ns], F16, kind="ExternalInput").ap()
    wts = nc.dram_tensor("wts", [D, 2 * D], F16, kind="ExternalInput").ap()
    consts = nc.dram_tensor("consts", [D, 2 * D], F32, kind="ExternalInput").ap()
    brow = nc.dram_tensor("brow", [1, 2 * D], F16, kind="ExternalInput").ap()
    dinv_c = nc.dram_tensor("dinv_c", [D, nt], F32, kind="ExternalInput").ap()
    sdeg_r = nc.dram_tensor("sdeg_r", [1, nt * TILE], F16, kind="ExternalInput").ap()
    idx_d = nc.dram_tensor("idx", [D, plan.idx_cols], I16, kind="ExternalInput").ap()
    doff_d = nc.dram_tensor("doff", [D, plan.ch_cols], F16, kind="ExternalInput").ap()
    out_d = nc.dram_tensor("out", [nt * D, D], F16, kind="ExternalOutput").ap()
    ncalls = plan.n_calls_padded
    cnts_d = nc.dram_tensor("cnts", [1, ncalls], mybir.dt.int32,
                            kind="ExternalInput").ap()

    # h stored in "vrow" tile layout: vrow u = p * nt + t holds the fp16
    # row of node t*128+p (within its shard). One contiguous DMA per layer
    # writes it, and gathers index vrows directly.
    hb = [nc.dram_tensor(f"h{i}b", [nt * D, D], F16).ap() for i in range(2)]
    hf = [nc.dram_tensor(f"h{i}f", [NCORES * nt * D, D], F16,
                         addr_space="Shared").ap()
          for i in range(2)]

    max_C = max(sum(plan.gb_C[g]) for g in range(len(plan.groups)))
    max_icols = max(sum(plan.gb_C[g]) * 8 for g in range(len(plan.groups)))

    with tile.TileContext(nc) as tc:
        with (
            tc.tile_pool(name="const", bufs=1) as cpool,
            tc.tile_pool(name="stage", bufs=5) as spool,
            tc.tile_pool(name="oh", bufs=5) as ohpool,
            tc.tile_pool(name="hbig", bufs=2) as bpool,
            tc.tile_pool(name="ev", bufs=4) as evpool,
            tc.tile_pool(name="acc", bufs=4, space="PSUM") as accpool,
            tc.tile_pool(name="ptr", bufs=2, space="PSUM") as trpool,
            tc.tile_pool(name="pd", bufs=2, space="PSUM") as pdpool,
        ):
            w_sb = cpool.tile([D, 2 * D], F16, tag="w")
            nc.sync.dma_start(out=w_sb[:], in_=wts[:])
            co_sb = cpool.tile([D, 2 * D], F32, tag="co")
            nc.sync.dma_start(out=co_sb[:], in_=consts[:])
            br_sb = cpool.tile([1, 2 * D], F16, tag="br")
            nc.sync.dma_start(out=br_sb[:], in_=brow[:])
            dv_sb = cpool.tile([D, nt], F32, tag="dv")
            nc.sync.dma_start(out=dv_sb[:], in_=dinv_c[:])
            sd_sb = cpool.tile([1, nt * TILE], F16, tag="sd")
            nc.sync.dma_start(out=sd_sb[:], in_=sdeg_r[:])

            xall = cpool.tile([D, ns], F16, tag="xall")
            nc.sync.dma_start(out=xall[:], in_=xT[:])
            cnt_sb = cpool.tile([1, ncalls], mybir.dt.int32, tag="cnts")
            nc.sync.dma_start(out=cnt_sb[:], in_=cnts_d[:])
            cregs = [nc.alloc_register(mybir.EngineType.Pool, f"gcnt{i}")
                     for i in range(8)]
            idx_all = cpool.tile([D, plan.idx_cols], I16, tag="idxall")
            nc.sync.dma_start(out=idx_all[:], in_=idx_d[:])
            do_all = cpool.tile([D, plan.ch_cols], F16, tag="doall")
            nc.sync.dma_start(out=do_all[:], in_=doff_d[:])

            W1 = w_sb[:, 0:D]
            W2 = w_sb[:, D:2 * D]
            iota = cpool.tile([D, D], F16, tag="iota16")
            nc.vector.tensor_copy(iota[:], co_sb[:, 0:D])
            ident = co_sb[:, D:2 * D]

            def tw(t):
                return TILE if t < nt - 1 else plan.last_w

            # ---- layer-1 dense: h0' = (x @ W1) * dinv ----
            hbig0 = bpool.tile([D, nt, D], F16, tag="hbig", name="hbig0")
            for t in range(nt):
                w = tw(t)
                pd = pdpool.tile([TILE, D], F32, tag="pd")
                nc.tensor.matmul(pd[:w, :], lhsT=xall[:, t * TILE:t * TILE + w],
                                 rhs=W1, start=True, stop=True)
                nc.scalar.activation(hbig0[:w, t, :], pd[:w, :], AF.Copy,
                                     scale=dv_sb[:w, t:t + 1])
            nc.sync.dma_start(out=hb[0][:], in_=hbig0[:])

            # ---- sparse layer (templated over layer index) ----
            max_Cgb = max((plan.gb_C[g][b] for g in range(len(plan.groups))
                           for b in range(nblk)), default=1)
            GMAX = 8  # dma_gather caps at ~1024 idxs (16KB desc ring)

            for _ in range(5):
                z = spool.tile([D, max_Cgb, TILE], F16, tag="stage")
                nc.vector.memset(z[:], 0.0)

            qctr = [0]
            callctr = [0]

            VR = 2 * nt * D  # vrows per 2-shard block

            def sparse_layer(li, dense_big=None, out_big=None):
                callctr[0] = 0
                src_full = hf[li]
                for g, tiles in enumerate(plan.groups):
                    Ctot = sum(plan.gb_C[g])
                    if Ctot == 0:
                        continue
                    accs = {}
                    started = set()
                    for t in tiles:
                        accs[t] = accpool.tile([TILE, D], F32, tag="acc", name=f"acc_t{t}")

                    gco = 0  # chunk offset within the group's doff columns
                    for b in range(nblk):
                        Cgb = plan.gb_C[g][b]
                        if Cgb == 0:
                            continue
                        ic0 = plan.idx_col[g][b]
                        stg = spool.tile([D, max_Cgb, TILE], F16, tag="stage")
                        oh_sb = ohpool.tile([D, max_Cgb, TILE], F16, tag="oh")
                        nc.vector.scalar_tensor_tensor(
                            out=oh_sb[:, :Cgb, :],
                            in0=do_all[:, plan.ch_col[g] + gco:
                                       plan.ch_col[g] + gco + Cgb].unsqueeze(2)
                                .broadcast_to([D, Cgb, TILE]),
                            scalar=1.0,
                            in1=iota.unsqueeze(1).broadcast_to([D, Cgb, TILE]),
                            op0=ALU.mult,
                            op1=ALU.is_equal,
                        )
                        k0 = 0
                        for (t, nch) in plan.gb_chunks[g][b]:
                            for c0 in range(0, nch, GMAX):
                                cn = min(GMAX, nch - c0)
                                ci = callctr[0]
                                if ci % 8 == 0:
                                    nc.gpsimd.reg_load(
                                        cregs, cnt_sb[0:1, ci:ci + 8])
                                nc.gpsimd.dma_gather(
                                    stg[:, k0 + c0:k0 + c0 + cn, :],
                                    src_full[b * VR:(b + 1) * VR, :],
                                    idx_all[:, ic0 + (k0 + c0) * 8:
                                            ic0 + (k0 + c0 + cn) * 8],
                                    cn * TILE,
                                    cregs[ci % 8],
                                    D,
                                    queue_num=qctr[0] % 4,
                                )
                                qctr[0] += 1
                                callctr[0] += 1
                            k0 += nch
                        k = 0
                        for (t, nch) in plan.gb_chunks[g][b]:
                            for _ in range(nch):
                                nc.tensor.matmul(
                                    accs[t][:], lhsT=oh_sb[:, k, :],
                                    rhs=stg[:, k, :],
                                    start=(t not in started), stop=False)
                                started.add(t)
                                k += 1
                        gco += Cgb

                    for t in tiles:
                        if t not in started:
                            continue
                        w = tw(t)
                        acc = accs[t]
                        # bias as rank-1: outer(sqrt(deg), b); sdeg rows
                        # beyond the tile width are zero-padded on the host
                        nc.tensor.matmul(
                            acc[:],
                            lhsT=sd_sb[:, t * TILE:(t + 1) * TILE],
                            rhs=br_sb[:, li * D:(li + 1) * D],
                            start=False, stop=True)
                        if li == 1:
                            nc.scalar.activation(out_big[:w, t, :], acc[:w, :],
                                                 AF.Copy,
                                                 scale=dv_sb[:w, t:t + 1])
                            continue
                        ev = evpool.tile([TILE, D], F32, tag="ev")
                        nc.scalar.activation(ev[:w, :], acc[:w, :], AF.Copy,
                                             scale=dv_sb[:w, t:t + 1])
                        if li == 0:
                            # fused layer-2 dense: h1' = (out1 @ W2) * dinv
                            ptr = trpool.tile([D, TILE], F32, tag="ptr")
                            nc.tensor.transpose(ptr[:, :w], ev[:w, :],
                                                ident[:w, :w])
                            trs = evpool.tile([D, TILE], F16, tag="trs")
                            nc.vector.tensor_copy(trs[:, :w], ptr[:, :w])
                            pd = pdpool.tile([TILE, D], F32, tag="pd")
                            nc.tensor.matmul(pd[:w, :], lhsT=trs[:, :w], rhs=W2,
                                             start=True, stop=True)
                            nc.scalar.activation(dense_big[:w, t, :], pd[:w, :],
                                                 AF.Copy,
                                                 scale=dv_sb[:w, t:t + 1])

            nc.gpsimd.collective_compute(
                "AllGather", ALU.bypass,
                replica_groups=[list(range(NCORES))],
                ins=[hb[0].opt()], outs=[hf[0].opt()])
            hbig1 = bpool.tile([D, nt, D], F16, tag="hbig", name="hbig1")
            sparse_layer(0, dense_big=hbig1)
            nc.sync.dma_start(out=hb[1][:], in_=hbig1[:])
            nc.gpsimd.collective_compute(
                "AllGather", ALU.bypass,
                replica_groups=[list(range(NCORES))],
                ins=[hb[1].opt()], outs=[hf[1].opt()])
            obig = bpool.tile([D, nt, D], F16, tag="hbig", name="obig")
            sparse_layer(1, out_big=obig)
            nc.sync.dma_start(out=out_d[:], in_=obig[:])

    nc.compile()
    return nc


def _install_ntff_hook():
    """antenv.axon_hooks is absent in this image; synthesize it and register
    the ctypes NTFF profile hook from the boot module."""
    import types
    if "antenv.axon_hooks" in sys.modules:
        return
    try:
        from trn_agent_boot.trn_boot import _ntff_profile_via_ctypes
        hook = _ntff_profile_via_ctypes("/opt/axon/libaxon_pjrt.so")
    except Exception as e:
        print(f"[kernel] ntff hook unavailable: {e}", flush=True)
        hook = None
    mod = types.ModuleType("antenv.axon_hooks")
    mod._hook = hook
    mod.set_axon_ntff_profile_hook = lambda h: setattr(mod, "_hook", h)
    mod.get_axon_ntff_profile_hook = lambda: mod._hook
    sys.modules["antenv.axon_hooks"] = mod
    import antenv
    antenv.axon_hooks = mod


def _run(plan, x, W1, b1, W2, b2, trace=False, stage="full"):
    import time
    if trace:
        _install_ntff_hook()
    t0 = time.time()
    nc = _build(plan, stage=stage)
    t1 = time.time()
    if os.environ.get("GCN_VERBOSE"):
        print(f"[kernel] build+compile: {t1 - t0:.1f}s", flush=True)
    ns, nt = plan.ns, plan.nt
    iota_t = np.tile(np.arange(TILE, dtype=np.float32), (TILE, 1))
    ident_t = np.eye(TILE, dtype=np.float32)
    consts = np.concatenate([iota_t, ident_t], axis=1)
    wts = np.concatenate([W1, W2], axis=1).astype(np.float16)
    brow = np.concatenate([b1, b2]).astype(np.float16).reshape(1, 2 * D)

    in_maps = []
    for c in range(NCORES):
        lo, hi = c * ns, (c + 1) * ns
        dv = plan.dinv[lo:hi]
        # column t of dcol holds dinv[lo + t*128 : lo + (t+1)*128] (pad 1.0)
        dcol = np.ones((nt, TILE), dtype=np.float32)
        dcol.reshape(-1)[:ns] = dv
        dcol = np.ascontiguousarray(dcol.T)
        sdr = np.zeros((1, nt * TILE), dtype=np.float16)
        sdr[0, :ns] = plan.sdeg[lo:hi].astype(np.float16)
        idx, doff = plan.core_inputs(c)
        cnts = plan.call_counts(c)
        assert cnts.shape[1] == plan.n_calls_padded, (cnts.shape, plan.n_calls_padded)
        in_maps.append({
            "cnts": cnts,
            "xT": np.ascontiguousarray(x[lo:hi].T.astype(np.float16)),
            "wts": wts, "consts": consts, "brow": brow,
            "dinv_c": dcol, "sdeg_r": sdr,
            "idx": idx, "doff": doff,
        })
    t2 = time.time()
    res = run_bass_kernel_spmd(nc, in_maps, core_ids=list(range(NCORES)),
                               trace=trace)
    if os.environ.get("GCN_VERBOSE"):
        print(f"[kernel] prep inputs: {t2 - t1:.1f}s, run: {time.time() - t2:.1f}s",
              flush=True)
    nt = plan.nt
    parts = []
    for c in range(NCORES):
        buf = res.results[c]["out"]  # [nt*128, 128] f16, vrow u = p*nt + t
        o = buf.reshape(128, nt, 128).transpose(1, 0, 2).reshape(nt * 128, 128)
        parts.append(o[:plan.ns].astype(np.float32))
    out = np.concatenate(parts, axis=0)
    return out, res


def kernel(x, edge_index, W1, b1, W2, b2):
    plan = Plan(x.shape[0], np.asarray(edge_index))
    out, _ = _run(plan, np.asarray(x), np.asarray(W1), np.asarray(b1),
                  np.asarray(W2), np.asarray(b2))
    return out

